# revision 36
# baseline (speedup 1.0000x reference)
"""Trainium2 Bass kernel v3 for nn_GATsimple (4-layer GAT + graph readout).

Key changes vs v2:
- One-hot St/S tiles precomputed on HOST, streamed from HBM per window
  (kills all IS_EQ vector work; frees dstb/dstp/iota SBUF).
- Attention logits assembled in PSUM by matmuls only: per tile
  pd = I@s_hi + I@s_lo + St@d_hi + St@d_lo  (s carried in gather rows,
  d per-window). Kills the strided s-extract and et adds on DVE.
- LeakyReLU (alpha=0.2) + Exp run on the scalar/ACT engine straight out
  of PSUM; Exp writes bf16 directly (no cast op).
- Message multiply batched per window (4 per-head DVE ops instead of
  per-tile).
- Gathers issued with prepare_only=True + trigger_dma: the Q7 only does
  descriptor-gen (~1.5us), transfers overlap each other and compute.
- Pad gather slots use index -1 (skipped by the ucode) -> ~11% fewer
  descriptors + bytes. First 4 hsrc pool slots memset once for safety.
- Node phase (normalize+bias+ELU) in bf16 (2x DVE rate).
- AllGather segments resized to [1152, 640, 384] rows and layer-0 AGs
  issued inside the a-phase loop, shrinking layer-boundary stalls.
"""

import os
import sys

import ml_dtypes
import numpy as np

for _p in ("/opt/trn_rl_repo", "/root/.axon_site/_ro/trn_rl_repo"):
    if os.path.isdir(_p) and _p not in sys.path:
        sys.path.append(_p)

import concourse.bass as bass
import concourse.bacc as bacc
import concourse.mybir as mybir
import concourse.tile as tile
from concourse.bass_utils import run_bass_kernel_spmd

F32 = mybir.dt.float32
BF16 = mybir.dt.bfloat16
I16 = mybir.dt.int16
I32 = mybir.dt.int32
U8 = mybir.dt.uint8

N_CORES = 8
HEADS = 4
PAD_CODE = 200  # dst code for pad slots: never matches one-hot rows 0..127
USE_PREP = os.environ.get("V3_PREP", "0") == "1"  # prep/trigger races on this stack
PAD_IDX = -1 if os.environ.get("V3_NEGPAD", "0") == "1" else 0  # -1 hangs ucode
CC_ON_VECTOR = os.environ.get("V3_CCVEC", "0") == "1"  # walrus rejects non-Pool CC

SEG_OFF = [0, 1664, 1920, 2176]  # 13/2/2 windows: big AG mid-layer, tiny tail AGs


class Cfg:
    def __init__(self, n_nodes, npg, in_feat, layer_out, n_cores=N_CORES):
        assert n_nodes % n_cores == 0
        self.n_nodes = n_nodes
        self.npg = npg
        self.n_cores = n_cores
        self.npc = n_nodes // n_cores
        self.nblk = (self.npc + 127) // 128
        self.npc_pad = self.nblk * 128
        self.nrows = n_cores * self.npc_pad
        self.in_feat = in_feat
        self.layer_out = layer_out
        self.f_out = [HEADS * c for c in layer_out]
        self.f_in = [in_feat] + self.f_out[:-1]
        self.n_layers = len(layer_out)
        self.gpc = self.npc // npg
        assert self.npc % npg == 0
        # table row width in bf16 elems; rows carry [h | s_hi(4) | s_lo(4)]
        self.row = []
        for l in range(self.n_layers):
            r = self.f_out[l] + 8
            r = ((r + 127) // 128) * 128  # gather elem_size: 256B granularity
            self.row.append(r)


def default_cfg():
    return Cfg(n_nodes=17024, npg=133, in_feat=64, layer_out=[128, 64, 32, 16])


# ------------------------------------------------------------ host preprocess


def preprocess_edges(cfg, edge_index):
    """Bucket real edges (no appended self-loops) by (core, window).

    Returns (tg, per_core): tg[g] = tiles in window g (incl. 1 self tile);
    per_core[c] = dict(gidx int16 [128, 8*tgat], oh bf16 [128, ttot*256])."""
    src = edge_index[0].astype(np.int64)
    dst = edge_index[1].astype(np.int64)
    core = dst // cfg.npc
    win = (dst % cfg.npc) // 128
    key = core * cfg.nblk + win
    order = np.argsort(key, kind="stable")
    src, dst, key = src[order], dst[order], key[order]
    nbuck = cfg.n_cores * cfg.nblk
    counts = np.bincount(key, minlength=nbuck)
    starts = np.concatenate([[0], np.cumsum(counts)])

    tg = []
    for g in range(cfg.nblk):
        m = max(int(counts[c * cfg.nblk + g]) for c in range(cfg.n_cores))
        tg.append(1 + max(1, (m + 127) // 128))
    ttot = sum(tg)

    # padded h_glob row index: 3 segments of [1152, 640, 384] rows per core
    loc = src % cfg.npc
    seg = np.where(loc < SEG_OFF[1], 0, np.where(loc < SEG_OFF[2], 1, 2))
    seg_base = np.array(
        [0, 8 * SEG_OFF[1], 8 * SEG_OFF[2]], dtype=np.int64
    )
    seg_off = np.array(SEG_OFF[:3], dtype=np.int64)
    seg_len = np.array(
        [SEG_OFF[1], SEG_OFF[2] - SEG_OFF[1], SEG_OFF[3] - SEG_OFF[2]],
        dtype=np.int64,
    )
    rpad = seg_base[seg] + (src // cfg.npc) * seg_len[seg] + (loc - seg_off[seg])
    dloc = (dst % cfg.npc) % 128

    iota128 = np.arange(128, dtype=np.int64)
    per_core = []
    for c in range(cfg.n_cores):
        gidx_cols, code_cols = [], []
        for g in range(cfg.nblk):
            b = c * cfg.nblk + g
            s0, s1 = starts[b], starts[b + 1]
            cnt = s1 - s0
            ngath = (tg[g] - 1) * 128
            sp = np.full(ngath, PAD_IDX, dtype=np.int64)
            sp[:cnt] = rpad[s0:s1]
            wrap = sp.astype(np.int16).reshape(-1, 16).T  # [16, ngath/16]
            gidx_cols.append(np.tile(wrap, (8, 1)))  # [128, ngath/16]
            codes = np.full(tg[g] * 128, PAD_CODE, dtype=np.int64)
            codes[0:128] = iota128  # self tile
            codes[128 : 128 + cnt] = dloc[s0:s1]
            code_cols.append(codes)
        codes_all = np.concatenate(code_cols)  # [ttot*128]
        # one-hot tiles: per tile t, cols 0:128 = St (St[p,c]=1 iff code[c]==p),
        # cols 128:256 = S = St^T (S[p,c]=1 iff code[p]==c)
        oh = np.zeros((128, ttot, 256), dtype=ml_dtypes.bfloat16)
        tt = np.repeat(np.arange(ttot), 128)
        cc = np.tile(iota128, ttot)
        m = codes_all < 128
        oh[codes_all[m], tt[m], cc[m]] = 1.0
        oh[cc[m], tt[m], 128 + codes_all[m]] = 1.0
        per_core.append(
            dict(
                gidx=np.ascontiguousarray(np.concatenate(gidx_cols, axis=1)),
                oh=np.ascontiguousarray(oh.reshape(128, ttot * 256)),
            )
        )
    return tg, per_core


def make_waug(W, a_s, a_d):
    fin, fout = W.shape
    H, C = a_s.shape
    assert H * C == fout
    A = np.zeros((fout, 2 * H), dtype=np.float64)
    for h in range(H):
        A[h * C : (h + 1) * C, h] = a_s[h]
        A[h * C : (h + 1) * C, H + h] = a_d[h]
    waug = np.concatenate([W.astype(np.float64), W.astype(np.float64) @ A], axis=1)
    return np.ascontiguousarray(waug.astype(ml_dtypes.bfloat16))


# ---------------------------------------------------------------- bass kernel


def build_kernel(cfg, tg):
    nblk = cfg.nblk
    ttot = sum(tg)  # total tiles incl self tiles
    tgat = ttot - nblk  # gathered tiles
    nc = bacc.Bacc(
        "TRN2", target_bir_lowering=False, debug=False, num_devices=cfg.n_cores
    )

    # ---- I/O
    xT0_d = nc.dram_tensor("xT0", [cfg.in_feat, cfg.npc_pad], BF16, kind="ExternalInput")
    waug_d, bias_d = [], []
    for l in range(cfg.n_layers):
        waug_d.append(
            nc.dram_tensor(
                f"waug{l}", [cfg.f_in[l], cfg.f_out[l] + 8], BF16, kind="ExternalInput"
            )
        )
        bias_d.append(
            nc.dram_tensor(f"bias{l}", [128, cfg.f_out[l]], F32, kind="ExternalInput")
        )
    gidx_d = nc.dram_tensor("gidx", [128, 8 * tgat], I16, kind="ExternalInput")
    oh_d = nc.dram_tensor("oh", [128, 256 * ttot], BF16, kind="ExternalInput")
    fcwn_d = nc.dram_tensor("fcwn", [cfg.npc_pad, 64], BF16, kind="ExternalInput")
    fcb_d = nc.dram_tensor("fcb", [1, 1], F32, kind="ExternalInput")
    y_d = nc.dram_tensor("y", [1, cfg.gpc], F32, kind="ExternalOutput")

    h_in, h_glob = [], []
    for l in range(cfg.n_layers):
        h_in.append(nc.dram_tensor(f"h_in{l}", [cfg.npc_pad, cfg.row[l]], BF16))
        h_glob.append(
            nc.dram_tensor(
                f"h_glob{l}", [cfg.nrows, cfg.row[l]], BF16, addr_space="Shared"
            )
        )
    p_dram = nc.dram_tensor("p_scratch", [cfg.npc_pad, 1], F32)

    ident_c = nc.inline_tensor(
        np.eye(128, dtype=np.float32).astype(ml_dtypes.bfloat16), name="ident_c"
    )

    rg = [list(range(cfg.n_cores))]
    SEG_BASE = [8 * o for o in SEG_OFF]

    with tile.TileContext(nc) as tc:
        dma_sem = nc.alloc_semaphore("gat_dma") if USE_PREP else None
        with (
            tc.tile_pool(name="persist", bufs=1) as pp,
            tc.tile_pool(name="work", bufs=2) as wp,
            tc.tile_pool(name="gather", bufs=5) as gp,
            tc.tile_pool(name="ohpool", bufs=4) as ohp,
            tc.tile_pool(name="xt", bufs=2) as xtp,
            tc.tile_pool(name="pesb", bufs=2) as psp,
            tc.tile_pool(name="pe_pool", bufs=2, space="PSUM") as pep,
            tc.tile_pool(name="pt_pool", bufs=2, space="PSUM") as ptp,
            tc.tile_pool(name="pd_pool", bufs=2, space="PSUM") as pdp,
        ):
            # ---- persistent loads
            ident_sb = pp.tile([128, 128], BF16, tag="ident")
            nc.sync.dma_start(ident_sb[:], ident_c[:])
            ones_sb = pp.tile([128, 1], F32, tag="ones")
            nc.vector.memset(ones_sb[:], 1.0)

            xT0_sb = pp.tile([cfg.in_feat, cfg.npc_pad], BF16, tag="xT0")
            nc.sync.dma_start(xT0_sb[:], xT0_d[:])
            waug_sb, bias_sb = [], []
            for l in range(cfg.n_layers):
                fin, fo = cfg.f_in[l], cfg.f_out[l]
                p = min(fin, 128)
                kt = (fin + 127) // 128
                w = pp.tile([p, kt, fo + 8], BF16, tag=f"waug{l}")
                nc.sync.dma_start(w[:], waug_d[l].rearrange("(kt p) f -> p kt f", p=p))
                waug_sb.append(w)
                b = pp.tile([128, fo], F32, tag=f"bias{l}")
                nc.sync.dma_start(b[:], bias_d[l][:])
                bias_sb.append(b)
            gidx_sb = pp.tile([128, 8 * tgat], I16, tag="gidx")
            nc.sync.dma_start(gidx_sb[:], gidx_d[:])
            fcw_sb = pp.tile([128, nblk, 64], BF16, tag="fcw")
            nc.sync.dma_start(fcw_sb[:], fcwn_d.rearrange("(b p) f -> p b f", p=128))
            fcb_sb = pp.tile([1, 1], F32, tag="fcb")
            nc.sync.dma_start(fcb_sb[:], fcb_d[:])
            p_sb = pp.tile([128, nblk], F32, tag="p_sb")

            def a_phase(l, g, hbuf, sw16, dw16, lhsT_fn):
                """h_aug = x @ waug for window g of layer l; fills hbuf bf16,
                sw16 = [s_hi|s_lo], dw16 = [d_hi|d_lo]; DMAs h_in[l] rows."""
                fin, fo = cfg.f_in[l], cfg.f_out[l]
                kt = (fin + 127) // 128
                ph = pep.tile([128, 2, 512], F32, tag="pe")
                for k in range(kt):
                    lh = lhsT_fn(k)
                    nc.tensor.matmul(
                        ph[:, 0, 0:fo], lhsT=lh, rhs=waug_sb[l][:, k, 0:fo],
                        start=(k == 0), stop=(k == kt - 1),
                    )
                    nc.tensor.matmul(
                        ph[:, 1, 0:8], lhsT=lh, rhs=waug_sb[l][:, k, fo : fo + 8],
                        start=(k == 0), stop=(k == kt - 1),
                    )
                nc.scalar.copy(hbuf[:, g, 0:fo], ph[:, 0, 0:fo])
                # hi/lo split of [s(4) | d(4)]: one ACT copy PSUM->SBUF f32,
                # then DVE ops stay off PSUM (no PE-write port contention)
                sd32 = wp.tile([128, 8], F32, tag="sd32")
                nc.scalar.copy(sd32[:], ph[:, 1, 0:8])
                tmp = wp.tile([128, 8], F32, tag="dtmp")
                nc.vector.tensor_copy(sw16[:, g, 0:4], sd32[:, 0:4])
                nc.vector.tensor_copy(dw16[:, g, 0:4], sd32[:, 4:8])
                nc.vector.tensor_copy(tmp[:, 0:4], sw16[:, g, 0:4])
                nc.vector.tensor_copy(tmp[:, 4:8], dw16[:, g, 0:4])
                nc.vector.tensor_tensor(
                    out=sw16[:, g, 4:8], in0=sd32[:, 0:4], in1=tmp[:, 0:4],
                    op=mybir.AluOpType.subtract,
                )
                nc.vector.tensor_tensor(
                    out=dw16[:, g, 4:8], in0=sd32[:, 4:8], in1=tmp[:, 4:8],
                    op=mybir.AluOpType.subtract,
                )
                # store table rows for window g: [h | s_hi | s_lo]
                # scalar HWDGE queue: decoupled from the oh-stream backlog on sync
                nc.scalar.dma_start(
                    h_in[l][g * 128 : (g + 1) * 128, 0:fo], hbuf[:, g, 0:fo]
                )
                nc.scalar.dma_start(
                    h_in[l][g * 128 : (g + 1) * 128, fo : fo + 8], sw16[:, g, :]
                )

            def do_allgather(l, seg):
                # issue from the vector queue: the gpsimd queue is saturated
                # with 9.4us blocking gathers, which delayed every AG trigger
                # to the end of the layer (measured: fired after 16/17 gathers)
                o0, o1 = SEG_OFF[seg], SEG_OFF[seg + 1]
                cc_engine = nc.vector if CC_ON_VECTOR else nc.gpsimd
                bass.BassGpSimd.collective_compute(
                    cc_engine,
                    "AllGather",
                    mybir.AluOpType.bypass,
                    replica_groups=rg,
                    ins=[h_in[l][o0:o1, :]],
                    outs=[h_glob[l][SEG_BASE[seg] : SEG_BASE[seg + 1], :]],
                )

            # ---- layer 0 A-phase over all windows, AGs interleaved
            hbuf_cur = xtp.tile([128, nblk, cfg.f_out[0]], BF16, tag="hbuf")
            sw16_cur = xtp.tile([128, nblk, 8], BF16, tag="sw16")
            dw16_cur = xtp.tile([128, nblk, 8], BF16, tag="dw16")
            for g in range(nblk):
                a_phase(
                    0, g, hbuf_cur, sw16_cur, dw16_cur,
                    lambda k, g=g: xT0_sb[:, g * 128 : (g + 1) * 128],
                )
                if g == 12:
                    do_allgather(0, 0)
                elif g == 14:
                    do_allgather(0, 1)
                elif g == nblk - 1:
                    do_allgather(0, 2)

            for l in range(cfg.n_layers):
                fo = cfg.f_out[l]
                C = fo // HEADS
                ROW = cfg.row[l]
                last = l == cfg.n_layers - 1
                if not last:
                    fo2 = cfg.f_out[l + 1]
                    kt_out = (fo + 127) // 128  # chunks of xT for layer l+1
                    xT_next = xtp.tile([min(128, fo), kt_out, cfg.npc_pad], BF16, tag="xT")
                    hbuf_next = xtp.tile([128, nblk, fo2], BF16, tag="hbuf")
                    sw16_next = xtp.tile([128, nblk, 8], BF16, tag="sw16")
                    dw16_next = xtp.tile([128, nblk, 8], BF16, tag="dw16")

                def node_phase(g, pesb):
                    rec = wp.tile([128, 4], F32, tag="rec")
                    nc.vector.tensor_scalar(
                        out=rec[:], in0=pesb[:, fo : fo + 4], scalar1=1e-30,
                        scalar2=None, op0=mybir.AluOpType.add,
                    )
                    nc.vector.reciprocal(rec[:], rec[:])
                    # normalize + bias per head (f32)
                    xp = wp.tile([128, fo], F32, tag="xp")
                    for h in range(HEADS):
                        nc.vector.scalar_tensor_tensor(
                            out=xp[:, h * C : (h + 1) * C],
                            in0=pesb[:, h * C : (h + 1) * C],
                            scalar=rec[:, h : h + 1],
                            in1=bias_sb[l][:, h * C : (h + 1) * C],
                            op0=mybir.AluOpType.mult,
                            op1=mybir.AluOpType.add,
                        )
                    # ELU: xn = max(exp(min(x,0)) - 1, x)
                    xm = wp.tile([128, fo], F32, tag="xm")
                    nc.vector.tensor_scalar(
                        out=xm[:], in0=xp[:], scalar1=0.0, scalar2=None,
                        op0=mybir.AluOpType.min,
                    )
                    nc.scalar.activation(
                        out=xm[:], in_=xm[:], func=mybir.ActivationFunctionType.Exp
                    )
                    xn = wp.tile([128, fo], BF16, tag="xn")
                    nc.vector.scalar_tensor_tensor(
                        out=xn[:], in0=xm[:], scalar=-1.0, in1=xp[:],
                        op0=mybir.AluOpType.add, op1=mybir.AluOpType.max,
                    )
                    if not last:
                        for fb in range(kt_out):
                            w = min(128, fo - fb * 128)
                            pt = ptp.tile([128, 128], BF16, tag="pt")
                            nc.tensor.transpose(
                                pt[0:w, :], xn[:, fb * 128 : fb * 128 + w],
                                ident_sb[:],
                            )
                            nc.scalar.copy(
                                xT_next[0:w, fb, g * 128 : (g + 1) * 128], pt[0:w, :]
                            )
                        a_phase(
                            l + 1, g, hbuf_next, sw16_next, dw16_next,
                            lambda k, g=g: xT_next[:, k, g * 128 : (g + 1) * 128],
                        )
                    else:
                        junk = wp.tile([128, 64], F32, tag="junk")
                        nc.vector.scalar_tensor_tensor(
                            out=junk[:], in0=xn[:, 0:64], scalar=1.0,
                            in1=fcw_sb[:, g, :],
                            op0=mybir.AluOpType.mult, op1=mybir.AluOpType.mult,
                            accum_out=p_sb[:, g : g + 1],
                        )

                pending = None
                toff = 0  # tile offset incl self tiles
                goff = 0  # gathered-tile offset (gidx)
                for g in range(nblk):
                    T = tg[g]
                    # ---- one-hot stream for this window
                    oh_sb = ohp.tile([128, T, 256], BF16, tag="oh")
                    nc.sync.dma_start(
                        oh_sb[:].rearrange("p t c -> p (t c)"),
                        oh_d[:, 256 * toff : 256 * (toff + T)],
                    )
                    # ---- gather + self tile; -1 pads are skipped by the
                    # ucode, so zero the pool slots once (first layer) to keep
                    # never-written pad rows finite (codes drop them later)
                    hsrc = gp.tile([128, T, ROW], BF16, tag="hsrc")
                    if l == 0 and g < 5 and PAD_IDX < 0:
                        nc.vector.memset(hsrc[:], 0.0)
                    nc.scalar.copy(hsrc[:, 0, 0:fo], hbuf_cur[:, g, 0:fo])
                    if USE_PREP:
                        nc.gpsimd.dma_gather(
                            out_ap=hsrc[:, 1:T, :],
                            in_ap=h_glob[l][:],
                            idxs_ap=gidx_sb[:, 8 * goff : 8 * (goff + T - 1)],
                            num_idxs=(T - 1) * 128,
                            num_idxs_reg=(T - 1) * 128,
                            elem_size=ROW,
                            single_packet=False,
                            prepare_only=True,
                            sem=dma_sem,
                        )
                        nc.gpsimd.trigger_dma(count=None)
                    else:
                        nc.gpsimd.dma_gather(
                            out_ap=hsrc[:, 1:T, :],
                            in_ap=h_glob[l][:],
                            idxs_ap=gidx_sb[:, 8 * goff : 8 * (goff + T - 1)],
                            num_idxs=(T - 1) * 128,
                            num_idxs_reg=(T - 1) * 128,
                            elem_size=ROW,
                            single_packet=False,
                        )
                    # ---- pass 1: logits in PSUM via matmuls only
                    # per-tile contiguous accumulation group:
                    # pd[:,t,:] = I@s_hi + I@s_lo + St@d_hi + St@d_lo
                    pd = pdp.tile([128, T, 4], F32, tag="pd")
                    for t in range(T):
                        s_hi = (
                            sw16_cur[:, g, 0:4] if t == 0
                            else hsrc[:, t, fo : fo + 4]
                        )
                        s_lo = (
                            sw16_cur[:, g, 4:8] if t == 0
                            else hsrc[:, t, fo + 4 : fo + 8]
                        )
                        nc.tensor.matmul(
                            pd[:, t, :], lhsT=ident_sb[:], rhs=s_hi,
                            start=True, stop=False,
                        )
                        nc.tensor.matmul(
                            pd[:, t, :], lhsT=ident_sb[:], rhs=s_lo,
                            start=False, stop=False,
                        )
                        nc.tensor.matmul(
                            pd[:, t, :], lhsT=oh_sb[:, t, 0:128],
                            rhs=dw16_cur[:, g, 0:4],
                            start=False, stop=False,
                        )
                        nc.tensor.matmul(
                            pd[:, t, :], lhsT=oh_sb[:, t, 0:128],
                            rhs=dw16_cur[:, g, 4:8],
                            start=False, stop=True,
                        )
                    # ---- LeakyReLU (ACT copy + DVE mult/max) + Exp (ACT)
                    pdf = pd[:].rearrange("p t f -> p (t f)")
                    etf = wp.tile([128, T * 4], F32, tag="etf")
                    nc.scalar.copy(etf[:], pdf)
                    nc.vector.scalar_tensor_tensor(
                        out=etf[:], in0=etf[:], scalar=0.2, in1=etf[:],
                        op0=mybir.AluOpType.mult, op1=mybir.AluOpType.max,
                    )
                    eeb = wp.tile([128, T * 4], BF16, tag="eeb")
                    nc.scalar.activation(
                        out=eeb[:], in_=etf[:],
                        func=mybir.ActivationFunctionType.Exp,
                    )
                    # ---- messages: per-head batched multiply over the window
                    msg = wp.tile([128, T, fo], BF16, tag="msg")
                    eb = eeb[:]
                    for h in range(HEADS):
                        ee_h = bass.AP(
                            eb.tensor, eb.offset + h,
                            [list(eb.ap[0]), [4, T], [0, C]],
                        )
                        nc.vector.tensor_tensor(
                            out=msg[:, :, h * C : (h + 1) * C],
                            in0=hsrc[:, :, h * C : (h + 1) * C],
                            in1=ee_h,
                            op=mybir.AluOpType.mult,
                        )
                    # ---- pass 2: aggregation matmuls
                    pe = pep.tile([128, 2, 512], F32, tag="pe")
                    for t in range(T):
                        S_t = oh_sb[:, t, 128:256]
                        nc.tensor.matmul(
                            pe[:, 0, 0:fo], lhsT=S_t, rhs=msg[:, t, :],
                            start=(t == 0), stop=(t == T - 1),
                        )
                        nc.tensor.matmul(
                            pe[:, 1, 0:4], lhsT=S_t, rhs=eeb[:, 4 * t : 4 * t + 4],
                            start=(t == 0), stop=(t == T - 1),
                        )
                    # ---- free PSUM early: copy aggregation to SBUF
                    pesb = psp.tile([128, fo + 4], F32, tag="pesb")
                    nc.scalar.copy(pesb[:, 0:fo], pe[:, 0, 0:fo])
                    nc.scalar.copy(pesb[:, fo : fo + 4], pe[:, 1, 0:4])
                    if pending is not None:
                        node_phase(pending[0], pending[1])
                        # AG triggers placed where their data-wait is ~zero so
                        # they don't stall the gather queue (gpsimd, in-order):
                        # seg0 (w0-12) after node_phase(13); seg1 (w13-14)
                        # after node_phase(14); seg2 (w15-16) post-loop
                        if not last:
                            if pending[0] == 13:
                                do_allgather(l + 1, 0)
                            elif pending[0] == 14:
                                do_allgather(l + 1, 1)
                    pending = (g, pesb)
                    toff += T
                    goff += T - 1

                node_phase(pending[0], pending[1])
                pending = None
                if not last:
                    do_allgather(l + 1, 2)

                if not last:
                    hbuf_cur, sw16_cur, dw16_cur = hbuf_next, sw16_next, dw16_next

            # ---- readout: per-graph sums of p over npg-node segments
            nc.sync.dma_start(
                p_dram.rearrange("(b p) one -> p (b one)", p=128), p_sb[:]
            )
            pw = min(128, cfg.npg)
            pa = pp.tile([pw, cfg.gpc], F32, tag="pa")
            pd_ap = p_dram[:]
            nc.sync.dma_start(
                pa[:], bass.AP(pd_ap.tensor, 0, [[1, pw], [cfg.npg, cfg.gpc]])
            )
            rem = cfg.npg - 128
            if rem > 0:
                pb = pp.tile([128, cfg.gpc], F32, tag="pb")
                nc.sync.dma_start(
                    pb[0:rem, :],
                    bass.AP(pd_ap.tensor, 128, [[1, rem], [cfg.npg, cfg.gpc]]),
                )
            yp = ptp.tile([1, cfg.gpc], F32, tag="pt")
            nc.tensor.matmul(
                yp[0:1, :], lhsT=ones_sb[0:pw, 0:1], rhs=pa[:],
                start=True, stop=(rem <= 0),
            )
            if rem > 0:
                nc.tensor.matmul(
                    yp[0:1, :], lhsT=ones_sb[0:rem, 0:1], rhs=pb[0:rem, :],
                    start=False, stop=True,
                )
            y_sb = pp.tile([1, cfg.gpc], F32, tag="y_sb")
            nc.vector.tensor_scalar(
                out=y_sb[:], in0=yp[0:1, :], scalar1=fcb_sb[0:1, 0:1], scalar2=None,
                op0=mybir.AluOpType.add,
            )
            nc.sync.dma_start(y_d[:], y_sb[:])

    nc.compile()
    return nc


# ------------------------------------------------------------------- driver

last_results = None
_cache = {}


def _prepare(cfg, inputs):
    tg, per_core = preprocess_edges(cfg, np.asarray(inputs["edge_index"]))
    x = np.asarray(inputs["x"], dtype=np.float32)
    fcw = np.asarray(inputs["fcw"], dtype=np.float32)
    fcb = np.asarray(inputs["fcb"], dtype=np.float32).reshape(1, 1)
    waugs, biases = [], []
    for l in range(cfg.n_layers):
        waugs.append(
            make_waug(
                np.asarray(inputs[f"W{l + 1}"], np.float32),
                np.asarray(inputs[f"as{l + 1}"], np.float32),
                np.asarray(inputs[f"ad{l + 1}"], np.float32),
            )
        )
        biases.append(
            np.ascontiguousarray(
                np.tile(np.asarray(inputs[f"b{l + 1}"], np.float32)[None, :], (128, 1))
            )
        )
    fcw_node_full = fcw.reshape(cfg.npg, 64)[np.arange(cfg.n_nodes) % cfg.npg]

    in_maps = []
    for c in range(cfg.n_cores):
        xs = x[c * cfg.npc : (c + 1) * cfg.npc]
        xT0 = np.zeros((cfg.in_feat, cfg.npc_pad), np.float32)
        xT0[:, : cfg.npc] = xs.T
        fcwn = np.zeros((cfg.npc_pad, 64), np.float32)
        fcwn[: cfg.npc] = fcw_node_full[c * cfg.npc : (c + 1) * cfg.npc]
        m = dict(
            xT0=np.ascontiguousarray(xT0.astype(ml_dtypes.bfloat16)),
            gidx=per_core[c]["gidx"],
            oh=per_core[c]["oh"],
            fcwn=np.ascontiguousarray(fcwn.astype(ml_dtypes.bfloat16)),
            fcb=fcb,
        )
        for l in range(cfg.n_layers):
            m[f"waug{l}"] = waugs[l]
            m[f"bias{l}"] = biases[l]
        in_maps.append(m)
    return tg, in_maps


def _ensure_ntff_hook():
    try:
        from antenv.axon_hooks import get_axon_ntff_profile_hook  # noqa: F401

        return
    except ImportError:
        pass
    try:
        import types

        import antenv

        mod = types.ModuleType("antenv.axon_hooks")
        holder = [None]
        mod.set_axon_ntff_profile_hook = lambda h: holder.__setitem__(0, h)
        mod.get_axon_ntff_profile_hook = lambda: holder[0]
        sys.modules["antenv.axon_hooks"] = mod
        antenv.axon_hooks = mod
        from trn_agent_boot.trn_boot import _ntff_profile_via_ctypes

        h = _ntff_profile_via_ctypes("/opt/axon/libaxon_pjrt.so")
        if h is not None:
            holder[0] = h
    except Exception:
        pass


def run(cfg, inputs, trace=False):
    global last_results
    if trace or os.environ.get("BASS_TRACE"):
        _ensure_ntff_hook()
    tg, in_maps = _prepare(cfg, inputs)
    key = (cfg.n_nodes, tuple(tg))
    if key not in _cache:
        _cache[key] = build_kernel(cfg, tg)
    nc = _cache[key]
    res = run_bass_kernel_spmd(
        nc, in_maps, core_ids=list(range(cfg.n_cores)), trace=trace
    )
    last_results = res
    y = np.concatenate([r["y"].reshape(-1) for r in res.results])
    return y.reshape(-1, 1).astype(np.float32)


def kernel(**inputs) -> np.ndarray:
    cfg = default_cfg()
    return run(cfg, inputs)


# revision 45
# speedup vs baseline: 1.0818x; 1.0818x over previous
"""Trainium2 Bass kernel v3 for nn_GATsimple (4-layer GAT + graph readout).

Key changes vs v2:
- One-hot St/S tiles precomputed on HOST, streamed from HBM per window
  (kills all IS_EQ vector work; frees dstb/dstp/iota SBUF).
- Attention logits assembled in PSUM by matmuls only: per tile
  pd = I@s_hi + I@s_lo + St@d_hi + St@d_lo  (s carried in gather rows,
  d per-window). Kills the strided s-extract and et adds on DVE.
- LeakyReLU (alpha=0.2) + Exp run on the scalar/ACT engine straight out
  of PSUM; Exp writes bf16 directly (no cast op).
- Message multiply batched per window (4 per-head DVE ops instead of
  per-tile).
- Gathers issued with prepare_only=True + trigger_dma: the Q7 only does
  descriptor-gen (~1.5us), transfers overlap each other and compute.
- Pad gather slots use index -1 (skipped by the ucode) -> ~11% fewer
  descriptors + bytes. First 4 hsrc pool slots memset once for safety.
- Node phase (normalize+bias+ELU) in bf16 (2x DVE rate).
- AllGather segments resized to [1152, 640, 384] rows and layer-0 AGs
  issued inside the a-phase loop, shrinking layer-boundary stalls.
"""

import os
import sys

import ml_dtypes
import numpy as np

for _p in ("/opt/trn_rl_repo", "/root/.axon_site/_ro/trn_rl_repo"):
    if os.path.isdir(_p) and _p not in sys.path:
        sys.path.append(_p)

import concourse.bass as bass
import concourse.bacc as bacc
import concourse.mybir as mybir
import concourse.tile as tile
from concourse.bass_utils import run_bass_kernel_spmd

F32 = mybir.dt.float32
BF16 = mybir.dt.bfloat16
I16 = mybir.dt.int16
I32 = mybir.dt.int32
U8 = mybir.dt.uint8

N_CORES = 8
HEADS = 4
PAD_CODE = 200  # dst code for pad slots: never matches one-hot rows 0..127
USE_PREP = os.environ.get("V3_PREP", "0") == "1"  # prep/trigger races on this stack
PAD_IDX = -1 if os.environ.get("V3_NEGPAD", "0") == "1" else 0  # -1 hangs ucode
CC_ON_VECTOR = os.environ.get("V3_CCVEC", "0") == "1"  # walrus rejects non-Pool CC
Q_SPLIT = os.environ.get("V3_Q2", "0") == "1"  # split gathers across 2 SWDGE queues

SEG_OFF = [0, 1152, 1792, 2176]


class Cfg:
    def __init__(self, n_nodes, npg, in_feat, layer_out, n_cores=N_CORES):
        assert n_nodes % n_cores == 0
        self.n_nodes = n_nodes
        self.npg = npg
        self.n_cores = n_cores
        self.npc = n_nodes // n_cores
        self.nblk = (self.npc + 127) // 128
        self.npc_pad = self.nblk * 128
        self.nrows = n_cores * self.npc_pad
        self.in_feat = in_feat
        self.layer_out = layer_out
        self.f_out = [HEADS * c for c in layer_out]
        self.f_in = [in_feat] + self.f_out[:-1]
        self.n_layers = len(layer_out)
        self.gpc = self.npc // npg
        assert self.npc % npg == 0
        # table row width in bf16 elems; rows carry [h | s_hi(4) | s_lo(4)]
        self.row = []
        for l in range(self.n_layers):
            r = self.f_out[l] + 8
            r = ((r + 127) // 128) * 128  # gather elem_size: 256B granularity
            self.row.append(r)


def default_cfg():
    return Cfg(n_nodes=17024, npg=133, in_feat=64, layer_out=[128, 64, 32, 16])


# ------------------------------------------------------------ host preprocess


def preprocess_edges(cfg, edge_index):
    """Bucket real edges (no appended self-loops) by (core, window).

    Returns (tg, per_core): tg[g] = tiles in window g (incl. 1 self tile);
    per_core[c] = dict(gidx int16 [128, 8*tgat], oh bf16 [128, ttot*256])."""
    src = edge_index[0].astype(np.int64)
    dst = edge_index[1].astype(np.int64)
    core = dst // cfg.npc
    win = (dst % cfg.npc) // 128
    key = core * cfg.nblk + win
    order = np.argsort(key, kind="stable")
    src, dst, key = src[order], dst[order], key[order]
    nbuck = cfg.n_cores * cfg.nblk
    counts = np.bincount(key, minlength=nbuck)
    starts = np.concatenate([[0], np.cumsum(counts)])

    tg = []
    for g in range(cfg.nblk):
        m = max(int(counts[c * cfg.nblk + g]) for c in range(cfg.n_cores))
        tg.append(1 + max(1, (m + 127) // 128))
    ttot = sum(tg)

    # padded h_glob row index: 3 segments of [1152, 640, 384] rows per core
    loc = src % cfg.npc
    seg = np.where(loc < SEG_OFF[1], 0, np.where(loc < SEG_OFF[2], 1, 2))
    seg_base = np.array(
        [0, 8 * SEG_OFF[1], 8 * SEG_OFF[2]], dtype=np.int64
    )
    seg_off = np.array(SEG_OFF[:3], dtype=np.int64)
    seg_len = np.array(
        [SEG_OFF[1], SEG_OFF[2] - SEG_OFF[1], SEG_OFF[3] - SEG_OFF[2]],
        dtype=np.int64,
    )
    rpad = seg_base[seg] + (src // cfg.npc) * seg_len[seg] + (loc - seg_off[seg])
    dloc = (dst % cfg.npc) % 128

    iota128 = np.arange(128, dtype=np.int64)
    per_core = []
    for c in range(cfg.n_cores):
        gidx_cols, code_cols = [], []
        for g in range(cfg.nblk):
            b = c * cfg.nblk + g
            s0, s1 = starts[b], starts[b + 1]
            cnt = s1 - s0
            ngath = (tg[g] - 1) * 128
            sp = np.full(ngath, PAD_IDX, dtype=np.int64)
            sp[:cnt] = rpad[s0:s1]
            wrap = sp.astype(np.int16).reshape(-1, 16).T  # [16, ngath/16]
            gidx_cols.append(np.tile(wrap, (8, 1)))  # [128, ngath/16]
            codes = np.full(tg[g] * 128, PAD_CODE, dtype=np.int64)
            codes[0:128] = iota128  # self tile
            codes[128 : 128 + cnt] = dloc[s0:s1]
            code_cols.append(codes)
        codes_all = np.concatenate(code_cols)  # [ttot*128]
        # one-hot tiles: per tile t, cols 0:128 = St (St[p,c]=1 iff code[c]==p),
        # cols 128:256 = S = St^T (S[p,c]=1 iff code[p]==c)
        oh = np.zeros((128, ttot, 256), dtype=ml_dtypes.bfloat16)
        tt = np.repeat(np.arange(ttot), 128)
        cc = np.tile(iota128, ttot)
        m = codes_all < 128
        oh[codes_all[m], tt[m], cc[m]] = 1.0
        oh[cc[m], tt[m], 128 + codes_all[m]] = 1.0
        per_core.append(
            dict(
                gidx=np.ascontiguousarray(np.concatenate(gidx_cols, axis=1)),
                oh=np.ascontiguousarray(oh.reshape(128, ttot * 256)),
            )
        )
    return tg, per_core


def make_waug(W, a_s, a_d):
    fin, fout = W.shape
    H, C = a_s.shape
    assert H * C == fout
    A = np.zeros((fout, 2 * H), dtype=np.float64)
    for h in range(H):
        A[h * C : (h + 1) * C, h] = a_s[h]
        A[h * C : (h + 1) * C, H + h] = a_d[h]
    waug = np.concatenate([W.astype(np.float64), W.astype(np.float64) @ A], axis=1)
    return np.ascontiguousarray(waug.astype(ml_dtypes.bfloat16))


# ---------------------------------------------------------------- bass kernel


def build_kernel(cfg, tg):
    nblk = cfg.nblk
    ttot = sum(tg)  # total tiles incl self tiles
    tgat = ttot - nblk  # gathered tiles
    nc = bacc.Bacc(
        "TRN2", target_bir_lowering=False, debug=False, num_devices=cfg.n_cores,
        num_swdge_queues=2 if Q_SPLIT else 1,
    )

    # ---- I/O
    xT0_d = nc.dram_tensor("xT0", [cfg.in_feat, cfg.npc_pad], BF16, kind="ExternalInput")
    waug_d, bias_d = [], []
    for l in range(cfg.n_layers):
        waug_d.append(
            nc.dram_tensor(
                f"waug{l}", [cfg.f_in[l], cfg.f_out[l] + 8], BF16, kind="ExternalInput"
            )
        )
        bias_d.append(
            nc.dram_tensor(f"bias{l}", [128, cfg.f_out[l]], F32, kind="ExternalInput")
        )
    gidx_d = nc.dram_tensor("gidx", [128, 8 * tgat], I16, kind="ExternalInput")
    oh_d = nc.dram_tensor("oh", [128, 256 * ttot], BF16, kind="ExternalInput")
    fcwn_d = nc.dram_tensor("fcwn", [cfg.npc_pad, 64], BF16, kind="ExternalInput")
    fcb_d = nc.dram_tensor("fcb", [1, 1], F32, kind="ExternalInput")
    y_d = nc.dram_tensor("y", [1, cfg.gpc], F32, kind="ExternalOutput")

    h_in, h_glob = [], []
    for l in range(cfg.n_layers):
        h_in.append(nc.dram_tensor(f"h_in{l}", [cfg.npc_pad, cfg.row[l]], BF16))
        h_glob.append(
            nc.dram_tensor(
                f"h_glob{l}", [cfg.nrows, cfg.row[l]], BF16, addr_space="Shared"
            )
        )
    p_dram = nc.dram_tensor("p_scratch", [cfg.npc_pad, 1], F32)

    ident_c = nc.inline_tensor(
        np.eye(128, dtype=np.float32).astype(ml_dtypes.bfloat16), name="ident_c"
    )

    rg = [list(range(cfg.n_cores))]
    SEG_BASE = [8 * o for o in SEG_OFF]

    with tile.TileContext(nc) as tc:
        dma_sem = nc.alloc_semaphore("gat_dma") if USE_PREP else None
        with (
            tc.tile_pool(name="persist", bufs=1) as pp,
            tc.tile_pool(name="work", bufs=2) as wp,
            tc.tile_pool(name="gather", bufs=5) as gp,
            tc.tile_pool(name="ohpool", bufs=3) as ohp,
            tc.tile_pool(name="xt", bufs=2) as xtp,
            tc.tile_pool(name="pesb", bufs=2) as psp,
            tc.tile_pool(name="pe_pool", bufs=2, space="PSUM") as pep,
            tc.tile_pool(name="pt_pool", bufs=2, space="PSUM") as ptp,
            tc.tile_pool(name="pd_pool", bufs=2, space="PSUM") as pdp,
        ):
            # ---- persistent loads
            ident_sb = pp.tile([128, 128], BF16, tag="ident")
            nc.sync.dma_start(ident_sb[:], ident_c[:])
            ones_sb = pp.tile([128, 1], F32, tag="ones")
            nc.vector.memset(ones_sb[:], 1.0)

            xT0_sb = pp.tile([cfg.in_feat, cfg.npc_pad], BF16, tag="xT0")
            nc.sync.dma_start(xT0_sb[:], xT0_d[:])
            waug_sb, bias_sb = [], []
            for l in range(cfg.n_layers):
                fin, fo = cfg.f_in[l], cfg.f_out[l]
                p = min(fin, 128)
                kt = (fin + 127) // 128
                w = pp.tile([p, kt, fo + 8], BF16, tag=f"waug{l}")
                nc.sync.dma_start(w[:], waug_d[l].rearrange("(kt p) f -> p kt f", p=p))
                waug_sb.append(w)
                b = pp.tile([128, fo], F32, tag=f"bias{l}")
                nc.sync.dma_start(b[:], bias_d[l][:])
                bias_sb.append(b)
            gidx_sb = pp.tile([128, 8 * tgat], I16, tag="gidx")
            nc.sync.dma_start(gidx_sb[:], gidx_d[:])
            fcw_sb = pp.tile([128, nblk, 64], BF16, tag="fcw")
            nc.sync.dma_start(fcw_sb[:], fcwn_d.rearrange("(b p) f -> p b f", p=128))
            fcb_sb = pp.tile([1, 1], F32, tag="fcb")
            nc.sync.dma_start(fcb_sb[:], fcb_d[:])
            p_sb = pp.tile([128, nblk], F32, tag="p_sb")

            def a_phase(l, g, hbuf, sw16, dw16, lhsT_fn):
                """h_aug = x @ waug for window g of layer l; fills hbuf bf16,
                sw16 = [s_hi|s_lo], dw16 = [d_hi|d_lo]; DMAs h_in[l] rows."""
                fin, fo = cfg.f_in[l], cfg.f_out[l]
                kt = (fin + 127) // 128
                ph = pep.tile([128, 2, 512], F32, tag="pe")
                for k in range(kt):
                    lh = lhsT_fn(k)
                    nc.tensor.matmul(
                        ph[:, 0, 0:fo], lhsT=lh, rhs=waug_sb[l][:, k, 0:fo],
                        start=(k == 0), stop=(k == kt - 1),
                    )
                    nc.tensor.matmul(
                        ph[:, 1, 0:8], lhsT=lh, rhs=waug_sb[l][:, k, fo : fo + 8],
                        start=(k == 0), stop=(k == kt - 1),
                    )
                nc.scalar.copy(hbuf[:, g, 0:fo], ph[:, 0, 0:fo])
                # hi/lo split of [s(4) | d(4)]: one ACT copy PSUM->SBUF f32,
                # then DVE ops stay off PSUM (no PE-write port contention)
                sd32 = wp.tile([128, 8], F32, tag="sd32")
                nc.scalar.copy(sd32[:], ph[:, 1, 0:8])
                tmp = wp.tile([128, 8], F32, tag="dtmp")
                nc.vector.tensor_copy(sw16[:, g, 0:4], sd32[:, 0:4])
                nc.vector.tensor_copy(dw16[:, g, 0:4], sd32[:, 4:8])
                nc.vector.tensor_copy(tmp[:, 0:4], sw16[:, g, 0:4])
                nc.vector.tensor_copy(tmp[:, 4:8], dw16[:, g, 0:4])
                nc.vector.tensor_tensor(
                    out=sw16[:, g, 4:8], in0=sd32[:, 0:4], in1=tmp[:, 0:4],
                    op=mybir.AluOpType.subtract,
                )
                nc.vector.tensor_tensor(
                    out=dw16[:, g, 4:8], in0=sd32[:, 4:8], in1=tmp[:, 4:8],
                    op=mybir.AluOpType.subtract,
                )
                # store table rows for window g: [h | s_hi | s_lo]
                nc.sync.dma_start(
                    h_in[l][g * 128 : (g + 1) * 128, 0:fo], hbuf[:, g, 0:fo]
                )
                nc.sync.dma_start(
                    h_in[l][g * 128 : (g + 1) * 128, fo : fo + 8], sw16[:, g, :]
                )

            def do_allgather(l, seg):
                # issue from the vector queue: the gpsimd queue is saturated
                # with 9.4us blocking gathers, which delayed every AG trigger
                # to the end of the layer (measured: fired after 16/17 gathers)
                o0, o1 = SEG_OFF[seg], SEG_OFF[seg + 1]
                cc_engine = nc.vector if CC_ON_VECTOR else nc.gpsimd
                bass.BassGpSimd.collective_compute(
                    cc_engine,
                    "AllGather",
                    mybir.AluOpType.bypass,
                    replica_groups=rg,
                    ins=[h_in[l][o0:o1, :]],
                    outs=[h_glob[l][SEG_BASE[seg] : SEG_BASE[seg + 1], :]],
                )

            # ---- layer 0 A-phase over all windows, AGs interleaved
            hbuf_cur = xtp.tile([128, nblk, cfg.f_out[0]], BF16, tag="hbuf")
            sw16_cur = xtp.tile([128, nblk, 8], BF16, tag="sw16")
            dw16_cur = xtp.tile([128, nblk, 8], BF16, tag="dw16")
            for g in range(nblk):
                a_phase(
                    0, g, hbuf_cur, sw16_cur, dw16_cur,
                    lambda k, g=g: xT0_sb[:, g * 128 : (g + 1) * 128],
                )
                if g == 8:
                    do_allgather(0, 0)
                elif g == 13:
                    do_allgather(0, 1)
                elif g == nblk - 1:
                    do_allgather(0, 2)

            for l in range(cfg.n_layers):
                fo = cfg.f_out[l]
                C = fo // HEADS
                ROW = cfg.row[l]
                last = l == cfg.n_layers - 1
                if not last:
                    fo2 = cfg.f_out[l + 1]
                    kt_out = (fo + 127) // 128  # chunks of xT for layer l+1
                    xT_next = xtp.tile([min(128, fo), kt_out, cfg.npc_pad], BF16, tag="xT")
                    hbuf_next = xtp.tile([128, nblk, fo2], BF16, tag="hbuf")
                    sw16_next = xtp.tile([128, nblk, 8], BF16, tag="sw16")
                    dw16_next = xtp.tile([128, nblk, 8], BF16, tag="dw16")

                def node_phase(g, pesb):
                    rec = wp.tile([128, 4], F32, tag="rec")
                    nc.vector.tensor_scalar(
                        out=rec[:], in0=pesb[:, fo : fo + 4], scalar1=1e-30,
                        scalar2=None, op0=mybir.AluOpType.add,
                    )
                    nc.vector.reciprocal(rec[:], rec[:])
                    # normalize + bias per head (f32)
                    xp = wp.tile([128, fo], F32, tag="xp")
                    for h in range(HEADS):
                        nc.vector.scalar_tensor_tensor(
                            out=xp[:, h * C : (h + 1) * C],
                            in0=pesb[:, h * C : (h + 1) * C],
                            scalar=rec[:, h : h + 1],
                            in1=bias_sb[l][:, h * C : (h + 1) * C],
                            op0=mybir.AluOpType.mult,
                            op1=mybir.AluOpType.add,
                        )
                    # ELU: xn = max(exp(min(x,0)) - 1, x)
                    xm = wp.tile([128, fo], F32, tag="xm")
                    nc.vector.tensor_scalar(
                        out=xm[:], in0=xp[:], scalar1=0.0, scalar2=None,
                        op0=mybir.AluOpType.min,
                    )
                    nc.scalar.activation(
                        out=xm[:], in_=xm[:], func=mybir.ActivationFunctionType.Exp
                    )
                    xn = wp.tile([128, fo], BF16, tag="xn")
                    nc.vector.scalar_tensor_tensor(
                        out=xn[:], in0=xm[:], scalar=-1.0, in1=xp[:],
                        op0=mybir.AluOpType.add, op1=mybir.AluOpType.max,
                    )
                    if not last:
                        for fb in range(kt_out):
                            w = min(128, fo - fb * 128)
                            pt = ptp.tile([128, 128], BF16, tag="pt")
                            nc.tensor.transpose(
                                pt[0:w, :], xn[:, fb * 128 : fb * 128 + w],
                                ident_sb[:],
                            )
                            nc.scalar.copy(
                                xT_next[0:w, fb, g * 128 : (g + 1) * 128], pt[0:w, :]
                            )
                        a_phase(
                            l + 1, g, hbuf_next, sw16_next, dw16_next,
                            lambda k, g=g: xT_next[:, k, g * 128 : (g + 1) * 128],
                        )
                        if g == 8:
                            do_allgather(l + 1, 0)
                        elif g == 13:
                            do_allgather(l + 1, 1)
                        elif g == nblk - 1:
                            do_allgather(l + 1, 2)
                    else:
                        junk = wp.tile([128, 64], F32, tag="junk")
                        nc.vector.scalar_tensor_tensor(
                            out=junk[:], in0=xn[:, 0:64], scalar=1.0,
                            in1=fcw_sb[:, g, :],
                            op0=mybir.AluOpType.mult, op1=mybir.AluOpType.mult,
                            accum_out=p_sb[:, g : g + 1],
                        )

                pending = None
                toff = 0  # tile offset incl self tiles
                goff = 0  # gathered-tile offset (gidx)
                for g in range(nblk):
                    T = tg[g]
                    # ---- one-hot stream for this window
                    oh_sb = ohp.tile([128, T, 256], BF16, tag="oh")
                    nc.sync.dma_start(
                        oh_sb[:].rearrange("p t c -> p (t c)"),
                        oh_d[:, 256 * toff : 256 * (toff + T)],
                    )
                    # ---- gather + self tile; -1 pads are skipped by the
                    # ucode, so zero the pool slots once (first layer) to keep
                    # never-written pad rows finite (codes drop them later)
                    hsrc = gp.tile([128, T, ROW], BF16, tag="hsrc")
                    if l == 0 and g < 5 and PAD_IDX < 0:
                        nc.vector.memset(hsrc[:], 0.0)
                    nc.scalar.copy(hsrc[:, 0, 0:fo], hbuf_cur[:, g, 0:fo])
                    if USE_PREP:
                        nc.gpsimd.dma_gather(
                            out_ap=hsrc[:, 1:T, :],
                            in_ap=h_glob[l][:],
                            idxs_ap=gidx_sb[:, 8 * goff : 8 * (goff + T - 1)],
                            num_idxs=(T - 1) * 128,
                            num_idxs_reg=(T - 1) * 128,
                            elem_size=ROW,
                            single_packet=False,
                            prepare_only=True,
                            sem=dma_sem,
                        )
                        nc.gpsimd.trigger_dma(count=None)
                    elif Q_SPLIT:
                        half = (T - 1) // 2
                        nc.gpsimd.dma_gather(
                            out_ap=hsrc[:, 1 : 1 + half, :],
                            in_ap=h_glob[l][:],
                            idxs_ap=gidx_sb[:, 8 * goff : 8 * (goff + half)],
                            num_idxs=half * 128,
                            num_idxs_reg=half * 128,
                            elem_size=ROW,
                            single_packet=False,
                            queue_num=0,
                        )
                        nc.gpsimd.dma_gather(
                            out_ap=hsrc[:, 1 + half : T, :],
                            in_ap=h_glob[l][:],
                            idxs_ap=gidx_sb[:, 8 * (goff + half) : 8 * (goff + T - 1)],
                            num_idxs=(T - 1 - half) * 128,
                            num_idxs_reg=(T - 1 - half) * 128,
                            elem_size=ROW,
                            single_packet=False,
                            queue_num=1,
                        )
                    else:
                        nc.gpsimd.dma_gather(
                            out_ap=hsrc[:, 1:T, :],
                            in_ap=h_glob[l][:],
                            idxs_ap=gidx_sb[:, 8 * goff : 8 * (goff + T - 1)],
                            num_idxs=(T - 1) * 128,
                            num_idxs_reg=(T - 1) * 128,
                            elem_size=ROW,
                            single_packet=False,
                        )
                    # ---- pass 1: logits in PSUM via matmuls only
                    # per-tile contiguous accumulation group:
                    # pd[:,t,:] = I@s_hi + I@s_lo + St@d_hi + St@d_lo
                    pd = pdp.tile([128, T, 4], F32, tag="pd")
                    for t in range(T):
                        s_hi = (
                            sw16_cur[:, g, 0:4] if t == 0
                            else hsrc[:, t, fo : fo + 4]
                        )
                        s_lo = (
                            sw16_cur[:, g, 4:8] if t == 0
                            else hsrc[:, t, fo + 4 : fo + 8]
                        )
                        nc.tensor.matmul(
                            pd[:, t, :], lhsT=ident_sb[:], rhs=s_hi,
                            start=True, stop=False,
                        )
                        nc.tensor.matmul(
                            pd[:, t, :], lhsT=ident_sb[:], rhs=s_lo,
                            start=False, stop=False,
                        )
                        nc.tensor.matmul(
                            pd[:, t, :], lhsT=oh_sb[:, t, 0:128],
                            rhs=dw16_cur[:, g, 0:4],
                            start=False, stop=False,
                        )
                        nc.tensor.matmul(
                            pd[:, t, :], lhsT=oh_sb[:, t, 0:128],
                            rhs=dw16_cur[:, g, 4:8],
                            start=False, stop=True,
                        )
                    # ---- LeakyReLU (ACT copy + DVE mult/max) + Exp (ACT)
                    pdf = pd[:].rearrange("p t f -> p (t f)")
                    etf = wp.tile([128, T * 4], F32, tag="etf")
                    nc.scalar.copy(etf[:], pdf)
                    nc.vector.scalar_tensor_tensor(
                        out=etf[:], in0=etf[:], scalar=0.2, in1=etf[:],
                        op0=mybir.AluOpType.mult, op1=mybir.AluOpType.max,
                    )
                    eeb = wp.tile([128, T * 4], BF16, tag="eeb")
                    nc.scalar.activation(
                        out=eeb[:], in_=etf[:],
                        func=mybir.ActivationFunctionType.Exp,
                    )
                    # ---- messages: per-head batched multiply over the window
                    msg = wp.tile([128, T, fo], BF16, tag="msg")
                    eb = eeb[:]
                    for h in range(HEADS):
                        ee_h = bass.AP(
                            eb.tensor, eb.offset + h,
                            [list(eb.ap[0]), [4, T], [0, C]],
                        )
                        nc.vector.tensor_tensor(
                            out=msg[:, :, h * C : (h + 1) * C],
                            in0=hsrc[:, :, h * C : (h + 1) * C],
                            in1=ee_h,
                            op=mybir.AluOpType.mult,
                        )
                    # ---- pass 2: aggregation matmuls
                    pe = pep.tile([128, 2, 512], F32, tag="pe")
                    for t in range(T):
                        S_t = oh_sb[:, t, 128:256]
                        nc.tensor.matmul(
                            pe[:, 0, 0:fo], lhsT=S_t, rhs=msg[:, t, :],
                            start=(t == 0), stop=(t == T - 1),
                        )
                        nc.tensor.matmul(
                            pe[:, 1, 0:4], lhsT=S_t, rhs=eeb[:, 4 * t : 4 * t + 4],
                            start=(t == 0), stop=(t == T - 1),
                        )
                    # ---- free PSUM early: copy aggregation to SBUF
                    pesb = psp.tile([128, fo + 4], F32, tag="pesb")
                    nc.scalar.copy(pesb[:, 0:fo], pe[:, 0, 0:fo])
                    nc.scalar.copy(pesb[:, fo : fo + 4], pe[:, 1, 0:4])
                    if pending is not None:
                        node_phase(pending[0], pending[1])
                    pending = (g, pesb)
                    toff += T
                    goff += T - 1

                node_phase(pending[0], pending[1])
                pending = None

                if not last:
                    hbuf_cur, sw16_cur, dw16_cur = hbuf_next, sw16_next, dw16_next

            # ---- readout: per-graph sums of p over npg-node segments
            nc.sync.dma_start(
                p_dram.rearrange("(b p) one -> p (b one)", p=128), p_sb[:]
            )
            pw = min(128, cfg.npg)
            pa = pp.tile([pw, cfg.gpc], F32, tag="pa")
            pd_ap = p_dram[:]
            nc.sync.dma_start(
                pa[:], bass.AP(pd_ap.tensor, 0, [[1, pw], [cfg.npg, cfg.gpc]])
            )
            rem = cfg.npg - 128
            if rem > 0:
                pb = pp.tile([128, cfg.gpc], F32, tag="pb")
                nc.sync.dma_start(
                    pb[0:rem, :],
                    bass.AP(pd_ap.tensor, 128, [[1, rem], [cfg.npg, cfg.gpc]]),
                )
            yp = ptp.tile([1, cfg.gpc], F32, tag="pt")
            nc.tensor.matmul(
                yp[0:1, :], lhsT=ones_sb[0:pw, 0:1], rhs=pa[:],
                start=True, stop=(rem <= 0),
            )
            if rem > 0:
                nc.tensor.matmul(
                    yp[0:1, :], lhsT=ones_sb[0:rem, 0:1], rhs=pb[0:rem, :],
                    start=False, stop=True,
                )
            y_sb = pp.tile([1, cfg.gpc], F32, tag="y_sb")
            nc.vector.tensor_scalar(
                out=y_sb[:], in0=yp[0:1, :], scalar1=fcb_sb[0:1, 0:1], scalar2=None,
                op0=mybir.AluOpType.add,
            )
            nc.sync.dma_start(y_d[:], y_sb[:])

    nc.compile()
    return nc


# ------------------------------------------------------------------- driver

last_results = None
_cache = {}


def _prepare(cfg, inputs):
    tg, per_core = preprocess_edges(cfg, np.asarray(inputs["edge_index"]))
    x = np.asarray(inputs["x"], dtype=np.float32)
    fcw = np.asarray(inputs["fcw"], dtype=np.float32)
    fcb = np.asarray(inputs["fcb"], dtype=np.float32).reshape(1, 1)
    waugs, biases = [], []
    for l in range(cfg.n_layers):
        waugs.append(
            make_waug(
                np.asarray(inputs[f"W{l + 1}"], np.float32),
                np.asarray(inputs[f"as{l + 1}"], np.float32),
                np.asarray(inputs[f"ad{l + 1}"], np.float32),
            )
        )
        biases.append(
            np.ascontiguousarray(
                np.tile(np.asarray(inputs[f"b{l + 1}"], np.float32)[None, :], (128, 1))
            )
        )
    fcw_node_full = fcw.reshape(cfg.npg, 64)[np.arange(cfg.n_nodes) % cfg.npg]

    in_maps = []
    for c in range(cfg.n_cores):
        xs = x[c * cfg.npc : (c + 1) * cfg.npc]
        xT0 = np.zeros((cfg.in_feat, cfg.npc_pad), np.float32)
        xT0[:, : cfg.npc] = xs.T
        fcwn = np.zeros((cfg.npc_pad, 64), np.float32)
        fcwn[: cfg.npc] = fcw_node_full[c * cfg.npc : (c + 1) * cfg.npc]
        m = dict(
            xT0=np.ascontiguousarray(xT0.astype(ml_dtypes.bfloat16)),
            gidx=per_core[c]["gidx"],
            oh=per_core[c]["oh"],
            fcwn=np.ascontiguousarray(fcwn.astype(ml_dtypes.bfloat16)),
            fcb=fcb,
        )
        for l in range(cfg.n_layers):
            m[f"waug{l}"] = waugs[l]
            m[f"bias{l}"] = biases[l]
        in_maps.append(m)
    return tg, in_maps


def _ensure_ntff_hook():
    try:
        from antenv.axon_hooks import get_axon_ntff_profile_hook  # noqa: F401

        return
    except ImportError:
        pass
    try:
        import types

        import antenv

        mod = types.ModuleType("antenv.axon_hooks")
        holder = [None]
        mod.set_axon_ntff_profile_hook = lambda h: holder.__setitem__(0, h)
        mod.get_axon_ntff_profile_hook = lambda: holder[0]
        sys.modules["antenv.axon_hooks"] = mod
        antenv.axon_hooks = mod
        from trn_agent_boot.trn_boot import _ntff_profile_via_ctypes

        h = _ntff_profile_via_ctypes("/opt/axon/libaxon_pjrt.so")
        if h is not None:
            holder[0] = h
    except Exception:
        pass


def run(cfg, inputs, trace=False):
    global last_results
    if trace or os.environ.get("BASS_TRACE"):
        _ensure_ntff_hook()
    tg, in_maps = _prepare(cfg, inputs)
    key = (cfg.n_nodes, tuple(tg))
    if key not in _cache:
        _cache[key] = build_kernel(cfg, tg)
    nc = _cache[key]
    res = run_bass_kernel_spmd(
        nc, in_maps, core_ids=list(range(cfg.n_cores)), trace=trace
    )
    last_results = res
    y = np.concatenate([r["y"].reshape(-1) for r in res.results])
    return y.reshape(-1, 1).astype(np.float32)


def kernel(**inputs) -> np.ndarray:
    cfg = default_cfg()
    return run(cfg, inputs)
